# revision 5
# baseline (speedup 1.0000x reference)
"""Trainium2 Bass kernel for nn_DAC_558345749225 (dynamic rotated grouped conv).

Reference (per sample b):
  pooled = mean_{H,W} x[b]                                  [C]
  angles = tanh(relu(pooled@W1^T+b1)@W2^T+b2) * pi/4        [G]
  rot[g] = bilinear-rotate(base_kernel[g], angles[g])       [Cg,Cg,3,3]
  feat   = grouped_conv3x3(x[b], rot, groups=G, pad=1)
  mod    = sigmoid(relu(pooled@M1^T+bm1)@M2^T+bm2)          [C]
  out    = feat * mod[:,None,None]

Sharding: data-parallel over batch — 2 samples per core on 8 cores.

Per-core algorithm (samples b=0,1; packs p=0,1 of 4 groups each):
  - x staged in zero-padded SBUF tiles [128ch, 66*66] declared float32r (raw
    fp32 bits; PE rounds internally, verified bitwise-identical to pre-round).
  - pooling: DVE reduce over the padded row (border zeros don't change sum);
    the 1/4096 scale is folded into the MLP weights host-side.
  - angle/gate MLPs as tiny PE matmuls (contraction chunks of 128) + ACT
    Relu/Tanh/Sigmoid; cos via Sin(x + pi/2).
  - kernel rotation is a linear map on the 9 taps: R[ij,mn](theta) built with
    ~24 elementwise DVE ops on a [16=(b,g), 81=(mn,ij)] layout, scattered into
    block-diag A [36,36] per (b,pack); then one fp32 matmul per (b,pack):
      out1[(g,ij),(ci,co)] = A^T @ Bmat   (Bmat = host-rearranged base_kernel)
  - out1 rows scattered via gpsimd casting DMAs (fp32 -> float32r) into
    block-diagonal conv weights lhsT [128=(g,ci), 9*128=(ij,(g,co))].
  - conv: per (b,pack), 8 chunks of 8 output rows; 9 shift matmuls (float32r,
    1 PE cycle/row) accumulate into a PSUM bank [128,512]; epilogue multiplies
    by the sigmoid gate (per-partition scalar on DVE) and DMAs to DRAM.
"""
import math
import numpy as np

N_CORES = 8
B, C, H, W = 16, 256, 64, 64
G, Cg = 8, 32
HID = 64
Bc = B // N_CORES          # samples per core = 2
NU = Bc * 2                # (b, pack) units per core = 4
HP, WP = H + 2, W + 2      # padded 66 x 66
NPIX = HP * WP             # 4356

_CACHE = {}


def _host_consts(base_kernel, ap_w1, ap_b1, ap_w2, ap_b2,
                 meta_w1, meta_b1, meta_w2, meta_b2):
    f32 = np.float32
    # Bmat [2, 36, 1024]: [p][g4*9+mn][ci*32+co] = base[4p+g4, co, ci, m, n]
    bk = np.asarray(base_kernel, f32)                      # [G, co, ci, 3, 3]
    bm = np.transpose(bk, (0, 3, 4, 2, 1))                 # [G, m, n, ci, co]
    bmat = np.ascontiguousarray(bm.reshape(2, 36, Cg * Cg))

    # R-build constants over free layout f = mn*9 + ij (mn-major)
    f = np.arange(81)
    mn, ij = f // 9, f % 9
    i, j = ij // 3, ij % 3
    m, n = mn // 3, mn % 3
    blocks = [
        (j - 1).astype(f32),                               # xx
        (i - 1).astype(f32),                               # yy
        (n == 0).astype(f32),                              # n0
        (n == 1).astype(f32),                              # n1
        (n == 1).astype(f32) - (n == 0).astype(f32),       # dn01
        (n == 2).astype(f32) - (n == 1).astype(f32),       # dn12
        (m == 0).astype(f32),                              # m0
        (m == 1).astype(f32),                              # m1
        (m == 1).astype(f32) - (m == 0).astype(f32),       # dm01
        (m == 2).astype(f32) - (m == 1).astype(f32),       # dm12
    ]
    consts = np.tile(np.concatenate(blocks)[None, :], (16, 1)).astype(f32)

    scale = f32(1.0 / (H * W))
    w1T = np.asarray(ap_w1, f32).T * scale                 # [256, 64]
    m1T = np.asarray(meta_w1, f32).T * scale
    mlp1 = np.ascontiguousarray(np.concatenate(
        [w1T[:128], w1T[128:], m1T[:128], m1T[128:]], axis=1), f32)  # [128,256]
    w2T = np.asarray(ap_w2, f32).T                         # [64, 8]
    m2T = np.asarray(meta_w2, f32).T                       # [64, 256]
    mlp2 = np.ascontiguousarray(np.concatenate([w2T, m2T], axis=1), f32)
    bias64 = np.ascontiguousarray(
        np.stack([np.asarray(ap_b1, f32), np.asarray(meta_b1, f32)], axis=1))
    b2v = np.asarray(ap_b2, f32).reshape(8, 1).copy()
    mb2v = np.ascontiguousarray(np.asarray(meta_b2, f32).reshape(2, 128).T)
    return dict(bmat=bmat, consts=consts, mlp1=mlp1, mlp2=mlp2,
                bias64=bias64, b2v=b2v, mb2v=mb2v)


def _build_nc():
    import concourse.tile as tile
    from concourse import bacc, mybir
    dt = mybir.dt
    AF = mybir.ActivationFunctionType
    OP = mybir.AluOpType

    nc = bacc.Bacc("TRN2", target_bir_lowering=False, debug=False,
                   enable_asserts=False, num_devices=N_CORES)

    xs = nc.dram_tensor("xs", [Bc, C, H, W], dt.float32r, kind="ExternalInput").ap()
    bmat_d = nc.dram_tensor("bmat", [2, 36, 1024], dt.float32, kind="ExternalInput").ap()
    consts_d = nc.dram_tensor("consts", [16, 810], dt.float32, kind="ExternalInput").ap()
    mlp1_d = nc.dram_tensor("mlp1", [128, 256], dt.float32, kind="ExternalInput").ap()
    mlp2_d = nc.dram_tensor("mlp2", [64, 264], dt.float32, kind="ExternalInput").ap()
    bias64_d = nc.dram_tensor("bias64", [64, 2], dt.float32, kind="ExternalInput").ap()
    b2v_d = nc.dram_tensor("b2v", [8, 1], dt.float32, kind="ExternalInput").ap()
    mb2v_d = nc.dram_tensor("mb2v", [128, 2], dt.float32, kind="ExternalInput").ap()
    y = nc.dram_tensor("y", [Bc, C, H, W], dt.float32, kind="ExternalOutput").ap()

    xs_flat = xs.rearrange("b c h w -> (b c) (h w)")
    y_flat = y.rearrange("b c h w -> (b c) (h w)")

    with tile.TileContext(nc) as tc:
        from contextlib import ExitStack
        ctx = ExitStack()
        cpool = ctx.enter_context(tc.tile_pool(name="cpool", bufs=1))
        xpool = ctx.enter_context(tc.tile_pool(name="xpool", bufs=NU))
        wpool = ctx.enter_context(tc.tile_pool(name="wpool", bufs=NU))
        o1pool = ctx.enter_context(tc.tile_pool(name="o1pool", bufs=2))
        apool = ctx.enter_context(tc.tile_pool(name="apool", bufs=2))
        outpool = ctx.enter_context(tc.tile_pool(name="outpool", bufs=6))
        pconv = ctx.enter_context(tc.tile_pool(name="pconv", bufs=4, space="PSUM"))
        prot = ctx.enter_context(tc.tile_pool(name="prot", bufs=2, space="PSUM"))
        pmlp = ctx.enter_context(tc.tile_pool(name="pmlp", bufs=2, space="PSUM"))

        # ---------- constants ----------
        consts_t = cpool.tile([16, 810], dt.float32)
        nc.sync.dma_start(consts_t[:], consts_d[:])
        XX, YY, N0, N1, DN01, DN12, M0, M1, DM01, DM12 = (
            consts_t[:, 81 * k:81 * (k + 1)] for k in range(10))
        mlp1_t = cpool.tile([128, 256], dt.float32)
        nc.sync.dma_start(mlp1_t[:], mlp1_d[:])
        mlp2_t = cpool.tile([64, 264], dt.float32)
        nc.sync.dma_start(mlp2_t[:], mlp2_d[:])
        bias64_t = cpool.tile([64, 2], dt.float32)
        nc.sync.dma_start(bias64_t[:], bias64_d[:])
        b2v_t = cpool.tile([8, 1], dt.float32)
        nc.sync.dma_start(b2v_t[:], b2v_d[:])
        mb2v_t = cpool.tile([128, 2], dt.float32)
        nc.sync.dma_start(mb2v_t[:], mb2v_d[:])
        bsb = cpool.tile([36, 2048], dt.float32)
        nc.sync.dma_start(bsb[:, 0:1024], bmat_d[0])
        nc.sync.dma_start(bsb[:, 1024:2048], bmat_d[1])

        # ---------- x load (zero-padded) + pooling ----------
        # pooled col layout: col = 2*pack + b  (so MLP contraction chunks are
        # contiguous column pairs)
        pooled = cpool.tile([128, NU], dt.float32)
        x_tiles = []
        for u in range(NU):
            b, p = divmod(u, 2)
            xt = xpool.tile([128, NPIX], dt.float32r)
            x3f = xt[:].bitcast(dt.float32).rearrange("c (h w) -> c h w", w=WP)
            nc.gpsimd.memset(x3f[:, 0, :], 0.0)
            nc.gpsimd.memset(x3f[:, HP - 1, :], 0.0)
            nc.gpsimd.memset(x3f[:, :, 0], 0.0)
            nc.gpsimd.memset(x3f[:, :, WP - 1], 0.0)
            src = xs_flat[b * C + 128 * p:b * C + 128 * (p + 1), :]
            nc.sync.dma_start(
                xt[:].rearrange("c (h w) -> c h w", w=WP)[:, 1:H + 1, 1:W + 1],
                src.rearrange("c (h w) -> c h w", w=W))
            nc.vector.reduce_sum(
                pooled[:, 2 * p + b:2 * p + b + 1],
                xt[:].bitcast(dt.float32),
                axis=mybir.AxisListType.X)
            x_tiles.append(xt)

        # ---------- angle + gate MLPs (both samples batched) ----------
        h_ps = pmlp.tile([64, 2], dt.float32, tag="mlp")
        nc.tensor.matmul(h_ps[:], mlp1_t[:, 0:64], pooled[:, 0:2], start=True, stop=False)
        nc.tensor.matmul(h_ps[:], mlp1_t[:, 64:128], pooled[:, 2:4], start=False, stop=True)
        h_sb = cpool.tile([64, 2], dt.float32)
        nc.scalar.activation(h_sb[:], h_ps[:], AF.Relu, bias=bias64_t[:, 0:1])

        ang_ps = pmlp.tile([8, 2], dt.float32, tag="mlp")
        nc.tensor.matmul(ang_ps[:], mlp2_t[:, 0:8], h_sb[:], start=True, stop=True)
        ang_t = cpool.tile([8, 2], dt.float32)
        nc.scalar.activation(ang_t[:], ang_ps[:], AF.Tanh, bias=b2v_t[:])
        ang_sb = cpool.tile([8, 2], dt.float32)
        nc.vector.tensor_scalar_mul(ang_sb[:], ang_t[:], math.pi / 4)
        halfpi = cpool.tile([8, 1], dt.float32)
        nc.gpsimd.memset(halfpi[:], math.pi / 2)
        c_sb = cpool.tile([8, 2], dt.float32)
        nc.scalar.activation(c_sb[:], ang_sb[:], AF.Sin, bias=halfpi[:])
        s_sb = cpool.tile([8, 2], dt.float32)
        nc.scalar.activation(s_sb[:], ang_sb[:], AF.Sin)
        c16 = cpool.tile([16, 1], dt.float32)
        s16 = cpool.tile([16, 1], dt.float32)
        for b in range(Bc):
            nc.sync.dma_start(c16[:][8 * b:8 * (b + 1), :], c_sb[:, b:b + 1])
            nc.sync.dma_start(s16[:][8 * b:8 * (b + 1), :], s_sb[:, b:b + 1])

        m_ps = pmlp.tile([64, 2], dt.float32, tag="mlp")
        nc.tensor.matmul(m_ps[:], mlp1_t[:, 128:192], pooled[:, 0:2], start=True, stop=False)
        nc.tensor.matmul(m_ps[:], mlp1_t[:, 192:256], pooled[:, 2:4], start=False, stop=True)
        m_sb = cpool.tile([64, 2], dt.float32)
        nc.scalar.activation(m_sb[:], m_ps[:], AF.Relu, bias=bias64_t[:, 1:2])
        mod_sb = cpool.tile([128, NU], dt.float32)   # col = 2*pack + b
        for p in range(2):
            mod_ps = pmlp.tile([128, 2], dt.float32, tag="mlp")
            nc.tensor.matmul(mod_ps[:], mlp2_t[:, 8 + 128 * p:8 + 128 * (p + 1)],
                             m_sb[:], start=True, stop=True)
            nc.scalar.activation(mod_sb[:, 2 * p:2 * (p + 1)], mod_ps[:],
                                 AF.Sigmoid, bias=mb2v_t[:, p:p + 1])

        # ---------- R matrices: [16=(8b+g), 81=(mn,ij)] ----------
        def vt(name):
            return cpool.tile([16, 81], dt.float32, name=f"rt_{name}")
        txc, tys, xr, av, fx = vt(1), vt(2), vt(3), vt(4), vt(5)
        txs, tyc, yr, bv, fy = vt(6), vt(7), vt(8), vt(9), vt(10)
        u0, u1, v0, v1, uu, vv, r_all = (vt(k) for k in range(11, 18))
        TT = nc.vector.tensor_tensor
        TS = nc.vector.tensor_scalar
        nc.vector.tensor_scalar_mul(txc[:], XX, c16[:])
        nc.vector.tensor_scalar_mul(tys[:], YY, s16[:])
        TT(xr[:], txc[:], tys[:], op=OP.add)
        TS(av[:], xr[:], 0.0, None, op0=OP.is_ge)
        TT(fx[:], xr[:], av[:], op=OP.subtract)
        nc.vector.tensor_scalar_add(fx[:], fx[:], 1.0)
        nc.vector.tensor_scalar_mul(txs[:], XX, s16[:])
        nc.vector.tensor_scalar_mul(tyc[:], YY, c16[:])
        TT(yr[:], tyc[:], txs[:], op=OP.subtract)
        TS(bv[:], yr[:], 0.0, None, op0=OP.is_ge)
        TT(fy[:], yr[:], bv[:], op=OP.subtract)
        nc.vector.tensor_scalar_add(fy[:], fy[:], 1.0)
        TT(u0[:], DN01, av[:], op=OP.mult)
        TT(u0[:], u0[:], N0, op=OP.add)
        TT(u1[:], DN12, av[:], op=OP.mult)
        TT(u1[:], u1[:], N1, op=OP.add)
        TT(v0[:], DM01, bv[:], op=OP.mult)
        TT(v0[:], v0[:], M0, op=OP.add)
        TT(v1[:], DM12, bv[:], op=OP.mult)
        TT(v1[:], v1[:], M1, op=OP.add)
        TT(uu[:], u1[:], u0[:], op=OP.subtract)
        TT(uu[:], uu[:], fx[:], op=OP.mult)
        TT(uu[:], uu[:], u0[:], op=OP.add)
        TT(vv[:], v1[:], v0[:], op=OP.subtract)
        TT(vv[:], vv[:], fy[:], op=OP.mult)
        TT(vv[:], vv[:], v0[:], op=OP.add)
        TT(r_all[:], uu[:], vv[:], op=OP.mult)

        # ---------- rotation matmul + weight scatter per (b, pack) ----------
        lhsT_tiles = []
        for u in range(NU):
            b, p = divmod(u, 2)
            a_t = apool.tile([36, 36], dt.float32)
            nc.gpsimd.memset(a_t[:], 0.0)
            for g4 in range(4):
                r = 8 * b + 4 * p + g4
                nc.sync.dma_start(
                    a_t[:][9 * g4:9 * (g4 + 1), 9 * g4:9 * (g4 + 1)],
                    r_all[:][r:r + 1].rearrange("q (mn ij) -> q mn ij", ij=9))
            o1_t = o1pool.tile([36, 1024], dt.float32)
            for hh in range(2):
                rot_ps = prot.tile([36, 512], dt.float32)
                nc.tensor.matmul(rot_ps[:], a_t[:],
                                 bsb[:, 1024 * p + 512 * hh:1024 * p + 512 * (hh + 1)],
                                 start=True, stop=True)
                nc.scalar.copy(o1_t[:, 512 * hh:512 * (hh + 1)], rot_ps[:])
            lt = wpool.tile([128, 9 * 128], dt.float32r)
            nc.gpsimd.memset(lt[:].bitcast(dt.float32), 0.0)
            for g4 in range(4):
                srcv = o1_t[:][9 * g4:9 * (g4 + 1)].rearrange(
                    "q (ci co) -> q ci co", co=32)
                for ij in range(9):
                    nc.gpsimd.dma_start(
                        lt[:][32 * g4:32 * (g4 + 1),
                              128 * ij + 32 * g4:128 * ij + 32 * (g4 + 1)],
                        srcv[ij:ij + 1])
            lhsT_tiles.append(lt)

        # ---------- conv + gate + store ----------
        NCH = 8          # output row-chunks per image (8 rows x 64 cols = 512)
        for u in range(NU):
            b, p = divmod(u, 2)
            x3 = x_tiles[u][:].rearrange("c (h w) -> c h w", w=WP)
            lt = lhsT_tiles[u]
            mod_col = mod_sb[:, 2 * p + b:2 * p + b + 1]
            for c8 in range(NCH):
                ps = pconv.tile([128, 512], dt.float32)
                for s in range(9):
                    ky, kx = divmod(s, 3)
                    rhs = x3[:, c8 * 8 + ky:c8 * 8 + ky + 8, kx:kx + W]
                    nc.tensor.matmul(ps[:], lt[:, 128 * s:128 * (s + 1)], rhs,
                                     start=(s == 0), stop=(s == 8))
                ot = outpool.tile([128, 512], dt.float32)
                nc.vector.tensor_scalar_mul(ot[:], ps[:], mod_col)
                nc.sync.dma_start(
                    y_flat[b * C + 128 * p:b * C + 128 * (p + 1),
                           512 * c8:512 * (c8 + 1)],
                    ot[:])
        ctx.close()

    nc.compile()
    return nc


def _get_nc():
    if "nc" not in _CACHE:
        _CACHE["nc"] = _build_nc()
    return _CACHE["nc"]


def run_on_device(inputs, trace=False, tmpdir=None):
    """Shard, run on 8 cores, gather. Returns (y_full, BassKernelResults)."""
    from concourse.bass_utils import run_bass_kernel_spmd
    x = np.ascontiguousarray(np.asarray(inputs["x"], np.float32))
    hc = _host_consts(
        inputs["base_kernel"], inputs["ap_w1"], inputs["ap_b1"],
        inputs["ap_w2"], inputs["ap_b2"], inputs["meta_w1"],
        inputs["meta_b1"], inputs["meta_w2"], inputs["meta_b2"])
    nc = _get_nc()
    in_maps = []
    for c in range(N_CORES):
        im = {"xs": np.ascontiguousarray(x[Bc * c:Bc * (c + 1)])}
        im.update(hc)
        in_maps.append(im)
    kw = {}
    if trace:
        kw = dict(trace=True, tmpdir=tmpdir)
    res = run_bass_kernel_spmd(nc, in_maps, core_ids=list(range(N_CORES)), **kw)
    y = np.concatenate([res.results[c]["y"] for c in range(N_CORES)], axis=0)
    return y, res


def kernel(**inputs):
    y, _ = run_on_device(inputs)
    return y
